# revision 11
# baseline (speedup 1.0000x reference)
"""Multi-head attention block (b=4, n=2048, d=256, h=8) on 8 TRN2 NeuronCores.

Sharding: core c handles (batch bi=c//2, query-half qh=c%2): it computes
K/V for the full sequence of its batch and Q for its 1024-row query half,
producing 1024 complete rows of the final output (host concatenates and
adds b_out; no cross-core reduction).

V2 design (all matmul operands fp16; PSUM fp32):
  - kT_all[hg] [128,2048]: 4 heads' K^T stacked (compact, whole-psum copies).
  - qT_pad[h] [128,1024]: per-head Q^T zero-padded to 128 partitions, so the
    scores matmul runs K=128 (lhsT = kT_all chunk; the zero rows of qT_pad
    mask the other heads). Single tile-position, full-array matmuls only.
  - q-chunks of 256: one scores psum tensor [128,4,256] (2 banks) holds all
    4 heads of a head-group for one k-tile; ONE exp [128,1024] per k-tile.
  - AV: [v|ones] lhsT [128,33] folds the softmax denominator (row 32);
    av accumulators [33,2,256] share a bank per head-pair -> psum fits in
    4 (scores) + 2 (av) + 2 (projection) = 8 banks.
  - QKV/V projection units are woven into the attention emission stream as
    PE filler to keep the tensor engine dense (HAM clock at 2+ GHz).
  - Normalization: denominator rows -> DRAM bounce -> batched reciprocal
    [128,32] -> broadcast-read -> DVE multiply -> outT (fp16).
"""
import numpy as np

import concourse.bacc as bacc
import concourse.bass as bass
import concourse.mybir as mybir
import concourse.tile as tile
from concourse.bass_utils import run_bass_kernel_spmd

F32 = mybir.dt.float32
F16 = mybir.dt.float16
Exp = mybir.ActivationFunctionType.Exp
Copy = mybir.ActivationFunctionType.Copy

B, N, D = 4, 2048, 256
H, DH = 8, 32
NQ = N // 2            # per-core query rows
SCALE = D ** -0.5      # 0.0625
NKT = N // 128         # 16 k-tiles
QC = 256               # q-chunk
NQC = NQ // QC         # 4 q-chunks per core

_BUILD_CACHE = {}


def build():
    if "nc" in _BUILD_CACHE:
        return _BUILD_CACHE["nc"]
    nc = bacc.Bacc()

    xT_d = nc.dram_tensor("xT", [D, N], F32, kind="ExternalInput")
    xqT_d = nc.dram_tensor("xqT", [D, NQ], F32, kind="ExternalInput")
    w_d = nc.dram_tensor("w_qkv", [D, 3 * D], F32, kind="ExternalInput")
    b_d = nc.dram_tensor("b_qkv", [1, 3 * D], F32, kind="ExternalInput")
    wo_d = nc.dram_tensor("w_out", [D, D], F32, kind="ExternalInput")
    out_d = nc.dram_tensor("out", [NQ, D], F32, kind="ExternalOutput")
    den_dram = nc.dram_tensor("den_scratch", [2, NQC, 2, 512], F32)
    recip_dram = nc.dram_tensor("recip_scratch", [2, NQC, 2, 512], F32)

    with tile.TileContext(nc) as tc:
        with (
            tc.tile_pool(name="persist", bufs=1) as persist,
            tc.tile_pool(name="probs", bufs=4) as prpool,
            tc.tile_pool(name="avsb", bufs=3) as avsb_pool,
            tc.tile_pool(name="norm", bufs=4) as norm_pool,
            tc.tile_pool(name="outsb", bufs=3) as out_pool,
            tc.tile_pool(name="kqps", bufs=2, space="PSUM") as kqps,
            tc.tile_pool(name="scps", bufs=2, space="PSUM") as scps,
            tc.tile_pool(name="avps", bufs=1, space="PSUM") as avps,
        ):
            # ---- persistent tiles / loads ----
            ones = persist.tile([1, 512], F16, name="ones")
            nc.vector.memset(ones, 1.0)

            w_sb = [persist.tile([128, 3 * D], F16, name=f"w{d2}") for d2 in range(2)]
            b_sb = persist.tile([1, 3 * D], F16, name="b_sb")
            xT_sb = [persist.tile([128, N], F16, name=f"xT{d2}") for d2 in range(2)]
            xqT_sb = [persist.tile([128, NQ], F16, name=f"xq{d2}") for d2 in range(2)]
            wo_sb = [persist.tile([128, D], F16, name=f"wo{g}") for g in range(2)]
            for d2 in range(2):
                nc.gpsimd.dma_start(out=w_sb[d2], in_=w_d[128 * d2:128 * (d2 + 1), :])
                nc.gpsimd.dma_start(out=xqT_sb[d2], in_=xqT_d[128 * d2:128 * (d2 + 1), :])
            nc.gpsimd.dma_start(out=b_sb, in_=b_d[:, :])
            for d2 in range(2):
                nc.gpsimd.dma_start(out=xT_sb[d2], in_=xT_d[128 * d2:128 * (d2 + 1), :])
            for g in range(2):
                nc.gpsimd.dma_start(out=wo_sb[g], in_=wo_d[128 * g:128 * (g + 1), :])

            # per-chunk tiles: a chunk is fully written before first read, so
            # tile-granular RAW tracking cannot create emission-order cycles
            kT_c = [[persist.tile([128, 512], F16, name=f"kT{g}_{c}")
                     for c in range(4)] for g in range(2)]
            qT_pad = [persist.tile([128, NQ], F16, name=f"qT{h}") for h in range(H)]
            v_st = [persist.tile([128, H * 33], F16, name=f"vst{s}")
                    for s in range(NKT)]
            outT_c = [[persist.tile([128, 256], F16, name=f"outT{g}_{c}")
                       for c in range(NQC)] for g in range(2)]
            for h in range(H):
                nc.vector.memset(qT_pad[h], 0.0)
            for s in range(NKT):
                nc.gpsimd.memset(v_st[s], 1.0)

            # ---- projection units (emitted woven into attention) ----
            def qT_unit(hg, c):
                """q^T for head-group hg, seq chunk c (512 wide)."""
                p = kqps.tile([128, 512], F32, tag="kq", name=f"kqq_{hg}_{c}")
                for d2 in range(2):
                    nc.tensor.matmul(
                        p[:, :], w_sb[d2][:, 128 * hg:128 * (hg + 1)],
                        xqT_sb[d2][:, 512 * c:512 * (c + 1)],
                        start=(d2 == 0), stop=False)
                nc.tensor.matmul(
                    p[:, :], b_sb[:, 128 * hg:128 * (hg + 1)], ones[:, :],
                    start=False, stop=True)
                for j in range(4):
                    dst = qT_pad[4 * hg + j][32 * j:32 * (j + 1),
                                             512 * c:512 * (c + 1)]
                    if j % 2 == 0:
                        nc.vector.tensor_copy(out=dst, in_=p[32 * j:32 * (j + 1), :])
                    else:
                        nc.scalar.activation(out=dst, in_=p[32 * j:32 * (j + 1), :],
                                             func=Copy)

            def kT_unit(hg, c):
                """k^T for head-group hg, seq chunk c (512 wide)."""
                p = kqps.tile([128, 512], F32, tag="kq", name=f"kqk_{hg}_{c}")
                for d2 in range(2):
                    nc.tensor.matmul(
                        p[:, :], w_sb[d2][:, D + 128 * hg:D + 128 * (hg + 1)],
                        xT_sb[d2][:, 512 * c:512 * (c + 1)],
                        start=(d2 == 0), stop=False)
                nc.tensor.matmul(
                    p[:, :], b_sb[:, D + 128 * hg:D + 128 * (hg + 1)], ones[:, :],
                    start=False, stop=True)
                nc.scalar.activation(out=kT_c[hg][c][:, :], in_=p[:, :], func=Copy)

            def v_unit(st):
                """v rows for seq tile st (128 wide), all 8 heads + ones col."""
                p = kqps.tile([128, D], F32, tag="kq", name=f"vv_{st}")
                for d2 in range(2):
                    nc.tensor.matmul(
                        p[:, :], xT_sb[d2][:, 128 * st:128 * (st + 1)],
                        w_sb[d2][:, 2 * D:3 * D],
                        start=(d2 == 0), stop=False)
                nc.tensor.matmul(
                    p[:, :], ones[:, :128], b_sb[:, 2 * D:3 * D],
                    start=False, stop=True)
                nc.vector.tensor_copy(
                    out=v_st[st].rearrange("p (h c) -> p h c", h=H)[:, :, 0:32],
                    in_=p.rearrange("p (h c) -> p h c", h=H))

            # ---- attention ----
            for hg in range(2):
                av_sb_all = {}
                for qc in range(NQC):
                    av4 = avps.tile([33, 4, 256], F32, tag="av",
                                    name=f"av_{hg}_{qc}")

                    def emit_av(pr, kt):
                        for j in range(4):
                            h = 4 * hg + j
                            # start=True clears has_written for the whole
                            # bank: only the first slice in each bank may
                            # issue it; its sibling inherits the clear.
                            nc.tensor.matmul(
                                av4[:, j, :],
                                v_st[kt][:, 33 * h:33 * h + 33],
                                pr[:, 256 * j:256 * (j + 1)],
                                start=(kt == 0 and j % 2 == 0),
                                stop=(kt == NKT - 1))

                    prev = None
                    for kt in range(NKT):
                        # ---- woven projection filler (PE stays dense) ----
                        if hg == 0 and qc == 0:
                            if kt == 0:
                                qT_unit(0, 0)
                                kT_unit(0, 0)
                            elif kt == 1:
                                qT_unit(0, 1)
                            elif kt % 4 == 0:
                                kT_unit(0, kt // 4)
                            v_unit(kt)
                        elif hg == 0 and qc == 1:
                            if kt in (0, 4):
                                qT_unit(1, kt // 4)
                            elif kt in (8, 12):
                                kT_unit(1, (kt - 8) // 4)
                        elif hg == 0 and qc == 2 and kt in (0, 4):
                            kT_unit(1, 2 + kt // 4)

                        S = scps.tile([128, 4, 256], F32, tag="S",
                                      name=f"S_{hg}_{qc}_{kt}")
                        for j in range(4):
                            nc.tensor.matmul(
                                S[:, j, :],
                                kT_c[hg][kt // 4][:, 128 * (kt % 4):128 * (kt % 4 + 1)],
                                qT_pad[4 * hg + j][:, QC * qc:QC * (qc + 1)],
                                start=True, stop=True)
                        pr = prpool.tile([128, 4 * QC], F16, tag="pr",
                                         name=f"pr_{hg}_{qc}_{kt}")
                        nc.scalar.activation(
                            out=pr, in_=S.rearrange("p a b -> p (a b)"),
                            func=Exp, scale=SCALE)
                        if prev is not None:
                            emit_av(prev, kt - 1)
                        prev = pr
                    emit_av(prev, NKT - 1)

                    a = avsb_pool.tile([33, 4, 256], F32, tag="avsb",
                                       name=f"avsb_{hg}_{qc}")
                    nc.vector.tensor_copy(a, av4[:, :, :])
                    nc.sync.dma_start(out=den_dram[hg, qc, :, :],
                                      in_=a[32:33, :, :])

                    # per-qc normalize: batched reciprocal [128, 8], one
                    # 4-head broadcast read, 4 muls (+ outproj when hg==1)
                    denb = norm_pool.tile([128, 8], F32, tag="denb",
                                          name=f"denb{hg}_{qc}")
                    nc.sync.dma_start(
                        out=denb,
                        in_=den_dram[hg, qc, :, :].rearrange("a c -> (a c)")
                        .rearrange("(p f) -> p f", p=128))
                    recb = norm_pool.tile([128, 8], F32, tag="recb",
                                          name=f"recb{hg}_{qc}")
                    nc.vector.reciprocal(recb, denb)
                    nc.sync.dma_start(
                        out=recip_dram[hg, qc, :, :].rearrange("a c -> (a c)")
                        .rearrange("(p f) -> p f", p=128),
                        in_=recb)
                    for p2 in range(2):
                        row = recip_dram[hg, qc, p2, :]
                        bc = norm_pool.tile([32, 512], F32, tag="bc",
                                            name=f"bc_{hg}_{qc}_{p2}")
                        nc.gpsimd.dma_start(
                            out=bc,
                            in_=bass.AP(tensor=row.tensor, offset=row.offset,
                                        ap=[[0, 32], row.ap[-1]]))
                        for i2 in range(2):
                            j = 2 * p2 + i2
                            nc.vector.tensor_mul(
                                outT_c[hg][qc][32 * j:32 * (j + 1), :],
                                a[0:32, j, :],
                                bc[:, 256 * i2:256 * (i2 + 1)])
                    if hg == 1:
                        for qt in (2 * qc, 2 * qc + 1):
                            po = kqps.tile([128, D], F32, tag="kq", name=f"po{qt}")
                            for g in range(2):
                                nc.tensor.matmul(
                                    po[:, :],
                                    outT_c[g][qt // 2][:, 128 * (qt % 2):128 * (qt % 2 + 1)],
                                    wo_sb[g][:, :],
                                    start=(g == 0), stop=(g == 1))
                            o = out_pool.tile([128, D], F32, tag="o", name=f"o{qt}")
                            nc.vector.tensor_copy(o, po[:, :])
                            nc.sync.dma_start(
                                out=out_d[128 * qt:128 * (qt + 1), :], in_=o)

    nc.compile()
    _BUILD_CACHE["nc"] = nc
    return nc


def _run(x, w_qkv, b_qkv, w_out, trace=False):
    nc = build()
    in_maps = []
    for c in range(8):
        bi, qh = c // 2, c % 2
        in_maps.append({
            "xT": np.ascontiguousarray(x[bi].T),
            "xqT": np.ascontiguousarray(x[bi, NQ * qh:NQ * (qh + 1)].T),
            "w_qkv": np.ascontiguousarray(w_qkv),
            "b_qkv": np.ascontiguousarray(b_qkv.reshape(1, 3 * D)),
            "w_out": np.ascontiguousarray(w_out),
        })
    res = run_bass_kernel_spmd(nc, in_maps, core_ids=list(range(8)), trace=trace)
    out = np.empty((B, N, D), dtype=np.float32)
    for c in range(8):
        bi, qh = c // 2, c % 2
        out[bi, NQ * qh:NQ * (qh + 1)] = res.results[c]["out"]
    return out, res


def kernel(x, w_qkv, b_qkv, w_out, b_out):
    x = np.asarray(x, dtype=np.float32)
    out, _ = _run(x, np.asarray(w_qkv, np.float32), np.asarray(b_qkv, np.float32),
                  np.asarray(w_out, np.float32))
    return out + np.asarray(b_out, np.float32)[None, None, :]


# revision 12
# speedup vs baseline: 1.1783x; 1.1783x over previous
"""Multi-head attention block (b=4, n=2048, d=256, h=8) on 8 TRN2 NeuronCores.

Sharding: core c handles (batch bi=c//2, query-half qh=c%2): it computes
K/V for the full sequence of its batch and Q for its 1024-row query half,
producing 1024 complete rows of the final output (host concatenates and
adds b_out; no cross-core reduction).

V2 design (all matmul operands fp16; PSUM fp32):
  - kT_all[hg] [128,2048]: 4 heads' K^T stacked (compact, whole-psum copies).
  - qT_pad[h] [128,1024]: per-head Q^T zero-padded to 128 partitions, so the
    scores matmul runs K=128 (lhsT = kT_all chunk; the zero rows of qT_pad
    mask the other heads). Single tile-position, full-array matmuls only.
  - q-chunks of 256: one scores psum tensor [128,4,256] (2 banks) holds all
    4 heads of a head-group for one k-tile; ONE exp [128,1024] per k-tile.
  - AV: [v|ones] lhsT [128,33] folds the softmax denominator (row 32);
    av accumulators [33,2,256] share a bank per head-pair -> psum fits in
    4 (scores) + 2 (av) + 2 (projection) = 8 banks.
  - QKV/V projection units are woven into the attention emission stream as
    PE filler to keep the tensor engine dense (HAM clock at 2+ GHz).
  - Normalization: denominator rows -> DRAM bounce -> batched reciprocal
    [128,32] -> broadcast-read -> DVE multiply -> outT (fp16).
"""
import numpy as np

import concourse.bacc as bacc
import concourse.bass as bass
import concourse.mybir as mybir
import concourse.tile as tile
from concourse.bass_utils import run_bass_kernel_spmd

F32 = mybir.dt.float32
F16 = mybir.dt.float16
Exp = mybir.ActivationFunctionType.Exp
Copy = mybir.ActivationFunctionType.Copy

B, N, D = 4, 2048, 256
H, DH = 8, 32
NQ = N // 2            # per-core query rows
SCALE = D ** -0.5      # 0.0625
NKT = N // 128         # 16 k-tiles
QC = 256               # q-chunk
NQC = NQ // QC         # 4 q-chunks per core

_BUILD_CACHE = {}


def build():
    if "nc" in _BUILD_CACHE:
        return _BUILD_CACHE["nc"]
    nc = bacc.Bacc()

    xT_d = nc.dram_tensor("xT", [D, N], F32, kind="ExternalInput")
    xqT_d = nc.dram_tensor("xqT", [D, NQ], F32, kind="ExternalInput")
    w_d = nc.dram_tensor("w_qkv", [D, 3 * D], F32, kind="ExternalInput")
    b_d = nc.dram_tensor("b_qkv", [1, 3 * D], F32, kind="ExternalInput")
    wo_d = nc.dram_tensor("w_out", [D, D], F32, kind="ExternalInput")
    out_d = nc.dram_tensor("out", [NQ, D], F32, kind="ExternalOutput")
    den_dram = nc.dram_tensor("den_scratch", [2, NQC, 2, 512], F32)
    recip_dram = nc.dram_tensor("recip_scratch", [2, NQC, 2, 512], F32)

    with tile.TileContext(nc) as tc:
        with (
            tc.tile_pool(name="persist", bufs=1) as persist,
            tc.tile_pool(name="probs", bufs=4) as prpool,
            tc.tile_pool(name="avsb", bufs=3) as avsb_pool,
            tc.tile_pool(name="norm", bufs=4) as norm_pool,
            tc.tile_pool(name="outsb", bufs=3) as out_pool,
            tc.tile_pool(name="kqps", bufs=2, space="PSUM") as kqps,
            tc.tile_pool(name="scps", bufs=2, space="PSUM") as scps,
            tc.tile_pool(name="avps", bufs=1, space="PSUM") as avps,
        ):
            # ---- persistent tiles / loads ----
            ones = persist.tile([1, 512], F16, name="ones")
            nc.vector.memset(ones, 1.0)

            w_sb = [persist.tile([128, 3 * D], F16, name=f"w{d2}") for d2 in range(2)]
            b_sb = persist.tile([1, 3 * D], F16, name="b_sb")
            xT_sb = [persist.tile([128, N], F16, name=f"xT{d2}") for d2 in range(2)]
            xqT_sb = [persist.tile([128, NQ], F16, name=f"xq{d2}") for d2 in range(2)]
            wo_sb = [persist.tile([128, D], F16, name=f"wo{g}") for g in range(2)]
            for d2 in range(2):
                nc.gpsimd.dma_start(out=w_sb[d2], in_=w_d[128 * d2:128 * (d2 + 1), :])
                nc.gpsimd.dma_start(out=xqT_sb[d2], in_=xqT_d[128 * d2:128 * (d2 + 1), :])
            nc.gpsimd.dma_start(out=b_sb, in_=b_d[:, :])
            for d2 in range(2):
                nc.gpsimd.dma_start(out=xT_sb[d2], in_=xT_d[128 * d2:128 * (d2 + 1), :])
            for g in range(2):
                nc.gpsimd.dma_start(out=wo_sb[g], in_=wo_d[128 * g:128 * (g + 1), :])

            # per-chunk tiles: a chunk is fully written before first read, so
            # tile-granular RAW tracking cannot create emission-order cycles
            kT_c = [[persist.tile([128, 512], F16, name=f"kT{g}_{c}")
                     for c in range(4)] for g in range(2)]
            qT_pad = [persist.tile([128, NQ], F16, name=f"qT{h}") for h in range(H)]
            v_st = [persist.tile([128, H * 33], F16, name=f"vst{s}")
                    for s in range(NKT)]
            outT_c = [[persist.tile([128, 256], F16, name=f"outT{g}_{c}")
                       for c in range(NQC)] for g in range(2)]
            for h in range(H):
                nc.gpsimd.memset(qT_pad[h], 0.0)
            for s in range(NKT):
                nc.gpsimd.memset(v_st[s], 1.0)

            # ---- projection units (emitted woven into attention) ----
            def qT_unit(hg, c):
                """q^T for head-group hg, seq chunk c (512 wide)."""
                p = kqps.tile([128, 512], F32, tag="kq", name=f"kqq_{hg}_{c}")
                for d2 in range(2):
                    nc.tensor.matmul(
                        p[:, :], w_sb[d2][:, 128 * hg:128 * (hg + 1)],
                        xqT_sb[d2][:, 512 * c:512 * (c + 1)],
                        start=(d2 == 0), stop=False)
                nc.tensor.matmul(
                    p[:, :], b_sb[:, 128 * hg:128 * (hg + 1)], ones[:, :],
                    start=False, stop=True)
                for j in range(4):
                    dst = qT_pad[4 * hg + j][32 * j:32 * (j + 1),
                                             512 * c:512 * (c + 1)]
                    if j % 2 == 0:
                        nc.vector.tensor_copy(out=dst, in_=p[32 * j:32 * (j + 1), :])
                    else:
                        nc.scalar.activation(out=dst, in_=p[32 * j:32 * (j + 1), :],
                                             func=Copy)

            def kT_unit(hg, c):
                """k^T for head-group hg, seq chunk c (512 wide)."""
                p = kqps.tile([128, 512], F32, tag="kq", name=f"kqk_{hg}_{c}")
                for d2 in range(2):
                    nc.tensor.matmul(
                        p[:, :], w_sb[d2][:, D + 128 * hg:D + 128 * (hg + 1)],
                        xT_sb[d2][:, 512 * c:512 * (c + 1)],
                        start=(d2 == 0), stop=False)
                nc.tensor.matmul(
                    p[:, :], b_sb[:, D + 128 * hg:D + 128 * (hg + 1)], ones[:, :],
                    start=False, stop=True)
                nc.scalar.activation(out=kT_c[hg][c][:, :], in_=p[:, :], func=Copy)

            def v_unit(st):
                """v rows for seq tile st (128 wide), all 8 heads + ones col."""
                p = kqps.tile([128, D], F32, tag="kq", name=f"vv_{st}")
                for d2 in range(2):
                    nc.tensor.matmul(
                        p[:, :], xT_sb[d2][:, 128 * st:128 * (st + 1)],
                        w_sb[d2][:, 2 * D:3 * D],
                        start=(d2 == 0), stop=False)
                nc.tensor.matmul(
                    p[:, :], ones[:, :128], b_sb[:, 2 * D:3 * D],
                    start=False, stop=True)
                nc.vector.tensor_copy(
                    out=v_st[st].rearrange("p (h c) -> p h c", h=H)[:, :, 0:32],
                    in_=p.rearrange("p (h c) -> p h c", h=H))

            # ---- attention ----
            for hg in range(2):
                av_sb_all = {}
                for qc in range(NQC):
                    av4 = avps.tile([33, 4, 256], F32, tag="av",
                                    name=f"av_{hg}_{qc}")

                    def emit_av(pr, kt):
                        for j in range(4):
                            h = 4 * hg + j
                            # start=True clears has_written for the whole
                            # bank: only the first slice in each bank may
                            # issue it; its sibling inherits the clear.
                            nc.tensor.matmul(
                                av4[:, j, :],
                                v_st[kt][:, 33 * h:33 * h + 33],
                                pr[:, 256 * j:256 * (j + 1)],
                                start=(kt == 0 and j % 2 == 0),
                                stop=(kt == NKT - 1))

                    prev = None
                    for kt in range(NKT):
                        # ---- woven projection filler (PE stays dense) ----
                        if hg == 0 and qc == 0:
                            if kt == 0:
                                qT_unit(0, 0)
                                kT_unit(0, 0)
                            elif kt == 1:
                                qT_unit(0, 1)
                            elif kt % 4 == 0:
                                kT_unit(0, kt // 4)
                            v_unit(kt)
                        elif hg == 0 and qc == 1:
                            if kt in (0, 4):
                                qT_unit(1, kt // 4)
                            elif kt in (8, 12):
                                kT_unit(1, (kt - 8) // 4)
                        elif hg == 0 and qc == 2 and kt in (0, 4):
                            kT_unit(1, 2 + kt // 4)

                        S = scps.tile([128, 4, 256], F32, tag="S",
                                      name=f"S_{hg}_{qc}_{kt}")
                        for j in range(4):
                            nc.tensor.matmul(
                                S[:, j, :],
                                kT_c[hg][kt // 4][:, 128 * (kt % 4):128 * (kt % 4 + 1)],
                                qT_pad[4 * hg + j][:, QC * qc:QC * (qc + 1)],
                                start=True, stop=True)
                        pr = prpool.tile([128, 4 * QC], F16, tag="pr",
                                         name=f"pr_{hg}_{qc}_{kt}")
                        nc.scalar.activation(out=pr, in_=S[:, :, :],
                                             func=Exp, scale=SCALE)
                        if prev is not None:
                            emit_av(prev, kt - 1)
                        prev = pr
                    emit_av(prev, NKT - 1)

                    a = avsb_pool.tile([33, 4, 256], F32, tag="avsb",
                                       name=f"avsb_{hg}_{qc}")
                    nc.vector.tensor_copy(a, av4[:, :, :])
                    nc.sync.dma_start(out=den_dram[hg, qc, :, :],
                                      in_=a[32:33, :, :])

                    # per-qc normalize: batched reciprocal [128, 8], one
                    # 4-head broadcast read, 4 muls (+ outproj when hg==1)
                    denb = norm_pool.tile([128, 8], F32, tag="denb",
                                          name=f"denb{hg}_{qc}")
                    nc.sync.dma_start(
                        out=denb,
                        in_=den_dram[hg, qc, :, :].rearrange("a c -> (a c)")
                        .rearrange("(p f) -> p f", p=128))
                    recb = norm_pool.tile([128, 8], F32, tag="recb",
                                          name=f"recb{hg}_{qc}")
                    nc.vector.reciprocal(recb, denb)
                    nc.sync.dma_start(
                        out=recip_dram[hg, qc, :, :].rearrange("a c -> (a c)")
                        .rearrange("(p f) -> p f", p=128),
                        in_=recb)
                    for p2 in range(2):
                        row = recip_dram[hg, qc, p2, :]
                        bc = norm_pool.tile([32, 512], F32, tag="bc",
                                            name=f"bc_{hg}_{qc}_{p2}")
                        nc.gpsimd.dma_start(
                            out=bc,
                            in_=bass.AP(tensor=row.tensor, offset=row.offset,
                                        ap=[[0, 32], row.ap[-1]]))
                        for i2 in range(2):
                            j = 2 * p2 + i2
                            nc.vector.tensor_mul(
                                outT_c[hg][qc][32 * j:32 * (j + 1), :],
                                a[0:32, j, :],
                                bc[:, 256 * i2:256 * (i2 + 1)])
                    if hg == 1:
                        for qt in (2 * qc, 2 * qc + 1):
                            po = kqps.tile([128, D], F32, tag="kq", name=f"po{qt}")
                            for g in range(2):
                                nc.tensor.matmul(
                                    po[:, :],
                                    outT_c[g][qt // 2][:, 128 * (qt % 2):128 * (qt % 2 + 1)],
                                    wo_sb[g][:, :],
                                    start=(g == 0), stop=(g == 1))
                            o = out_pool.tile([128, D], F32, tag="o", name=f"o{qt}")
                            nc.vector.tensor_copy(o, po[:, :])
                            nc.sync.dma_start(
                                out=out_d[128 * qt:128 * (qt + 1), :], in_=o)

    nc.compile()
    _BUILD_CACHE["nc"] = nc
    return nc


def _run(x, w_qkv, b_qkv, w_out, trace=False):
    nc = build()
    in_maps = []
    for c in range(8):
        bi, qh = c // 2, c % 2
        in_maps.append({
            "xT": np.ascontiguousarray(x[bi].T),
            "xqT": np.ascontiguousarray(x[bi, NQ * qh:NQ * (qh + 1)].T),
            "w_qkv": np.ascontiguousarray(w_qkv),
            "b_qkv": np.ascontiguousarray(b_qkv.reshape(1, 3 * D)),
            "w_out": np.ascontiguousarray(w_out),
        })
    res = run_bass_kernel_spmd(nc, in_maps, core_ids=list(range(8)), trace=trace)
    out = np.empty((B, N, D), dtype=np.float32)
    for c in range(8):
        bi, qh = c // 2, c % 2
        out[bi, NQ * qh:NQ * (qh + 1)] = res.results[c]["out"]
    return out, res


def kernel(x, w_qkv, b_qkv, w_out, b_out):
    x = np.asarray(x, dtype=np.float32)
    out, _ = _run(x, np.asarray(w_qkv, np.float32), np.asarray(b_qkv, np.float32),
                  np.asarray(w_out, np.float32))
    return out + np.asarray(b_out, np.float32)[None, None, :]
